# revision 46
# baseline (speedup 1.0000x reference)
"""HAttentionNetwork Trainium2 kernel (v8).

Strategy (8 NeuronCores, data-parallel over bags), single-shipment design:
- 4096 bags split into 80 contiguous chunks (10/core, <=64 bags each),
  balanced by sentence count; each chunk padded to tiles of 128 sentences.
- x ships ONCE per core, transposed (partition = hidden-half):
    xq [128, CH*2*Tc*128] bf16
    mt [128, CH*3*Tc]     f32   per-sentence (lbl1, lbl0, sg)
- Per 128-sentence tile on device:
    flt[s,c]  = x @ [rel0|rel1]^T          (PE, 2 bf16 mm, [128,67] psum)
    P[s,c']   = x @ disc^T (2 level halves) (PE, 2 bf16 mm, [128,106] psum)
    E = exp(flt)   batched 7 tiles/op       (ACT, psum->sbuf bf16)
    Psb = copy(P)  batched 4 tiles/op       (ACT, psum->sbuf bf16)
    et1 = E[s, 14+lbl]   (Pool scalar_tensor_tensor is_equal/mult + accum)
    et0 = E[s, lbl0]     (DVE scalar_tensor_tensor is_equal/mult + accum)
    a2[s, 64l+g] = (io64==sg) * et_l        (DVE tensor_scalar x2, 4x mode)
    O[128, 106]  += a2^T @ Psb              (PE accumulating over chunk)
    dn[128, 1]   += a2^T @ ones
- Chunk epilogue (pipelined 2 chunks behind): inv = 1/(dn+eps) (DVE);
  obo = O * inv (ACT activation with per-partition scale); both level
  blocks obo[0:64,0:53] and obo[64:128,53:106] DMA to DRAM.
- Host: gather chunks, sum the two level blocks, add bias.
HW notes baked in here: walrus allows 1 sem wait per instruction (extras
ride the paired Ldweights or seq NoOps); same-engine sem waits are
load-bearing on real HW (engines pass parked instructions); fp32 matmul
accumulation groups with partition-offset operands hang the device; two
concurrent PSUM accumulation groups must not share a bank.
Numerics: bf16 inputs/intermediates, fp32 PSUM accumulation, fp32 epilogue.
"""

import numpy as np

N_SENT = 262144
N_BAGS = 4096
HIDDEN = 256
L0 = 14
NCLS = 53
NCORE = 8
CHUNKS_PER_CORE = 10
NCHUNK = NCORE * CHUNKS_PER_CORE
MAX_BAGS_PER_CHUNK = 64
EXP_GROUP = 5      # tiles per exp batch ( 5*67 = 335 f32 <= 512 psum bank )
P_GROUP = 4        # tiles per P-copy batch ( 4*106 = 424 <= 512 )
DMA_SPLIT = 4
LAG = 12
TAILB = 8      # tiles per et/a2 batch allocation
DROP_SELF_WAITS = ()  # engine names whose self-waits are dropped
USE_ENG_NOPS = False
LITE_EPI = False
EPI_DEPTH = 2           # tiles between compute emit and tail emit

_CACHE = {}


def _patch_tile_drain():
    # This walrus build rejects Drain instructions carrying more than ~1 sync
    # wait. Split the Tile final-drain waits across SP nops, one wait each.
    import concourse.mybir as mybir
    import concourse.tile as tile_mod
    from concourse.vector_clock import ScopedClock

    if getattr(tile_mod.TileContext, "_drain_split_patched", False):
        return

    def _split_drain_and_barrier(self, tick_clock, wait_clock):
        drain_inst = self.nc.sync.drain()
        wait_clock.add_sem_waits(
            drain_inst.ins, ScopedClock({None: tick_clock.global_clock})
        )
        si = drain_inst.ins.sync_info
        waits = list(si.on_wait) if si is not None else []
        if len(waits) > 1:
            drain_inst.ins.sync_info = mybir.SyncInfo(
                on_wait=waits[:1], on_update=list(si.on_update)
            )
            for w in waits[1:]:
                nop = self.nc.sync.nop(nofuse=True, hint="drain_wait_split")
                nop.ins.sync_info = mybir.SyncInfo(on_wait=[w], on_update=[])
        self.nc.all_engine_barrier()
        assert self.sems is not None
        popped = self.nc._tile_sem_poison_stack.pop()
        assert popped is self._sem_poison
        self.nc.clear_and_free_semaphores(list(self.sems.allocated().values()))
        self.nc.all_engine_barrier()

    tile_mod.TileContext._drain_and_barrier = _split_drain_and_barrier
    tile_mod.TileContext._drain_split_patched = True


def _split_all_waits(nc, max_waits=1):
    """This walrus build caps sync-wait commands per instruction to one.
    Excess waits are donated to preceding same-engine compute instructions
    with spare wait slots (in an in-order engine stream a wait moved
    earlier is strictly conservative; producers trail by many tiles so no
    cycle can form within the short lookback window). Leftovers fall back
    to seq NoOps inserted just before the instruction."""
    import concourse.mybir as mybir

    # Engines whose ENGINE component executes strictly in order: a sem wait
    # on the instruction's own engine sem is implied by program order and
    # can be dropped. (DMA-completion sems have queue names, not engine
    # names, so they are never touched.)
    inorder = {
        mybir.EngineType.PE: "PE",
        mybir.EngineType.DVE: "DVE",
        mybir.EngineType.Pool: "Pool",
        mybir.EngineType.Activation: "Activation",
    }
    # Pool ENGINE_NOP pays the 95ns q7 launch on the engine; keep engine
    # nops for DVE only.
    eng_nop_ok = {
        mybir.EngineType.DVE,
    }
    nop_opc = nc.isa.Opcode.NEURON_ISA_TPB_OPCODE_ENGINE_NOP

    def make_nop(engine):
        # A real ENGINE_NOP executes on the engine: its sem wait rides the
        # engine wait-queue instead of blocking the sequencer (InstNoOp is
        # sequencer-only and stalls instruction issue while waiting).
        if engine in eng_nop_ok and USE_ENG_NOPS:
            return nc.engines[engine]._isa(nop_opc, {})
        nop = mybir.InstNoOp(name=f"waitsplit-{n}", ins=[], outs=[])
        nop.engine = engine
        return nop

    n = 0
    for f in nc.m.functions:
        for b in f.blocks:
            new = []
            for inst in b.instructions:
                si = getattr(inst, "sync_info", None)
                waits = list(si.on_wait) if si is not None else []
                pfx = inorder.get(inst.engine)
                if pfx not in DROP_SELF_WAITS:
                    pfx = None
                if pfx is not None and waits:
                    kept = [
                        w for w in waits
                        if not (
                            getattr(w, "sync_type", "") == "semaphore"
                            and str(getattr(w, "ant_name", "")).split("_")[0]
                            == pfx
                        )
                    ]
                    if len(kept) != len(waits):
                        waits = kept
                        inst.sync_info = mybir.SyncInfo(
                            on_wait=waits, on_update=list(si.on_update)
                        )
                if len(waits) > max_waits:
                    keep = waits[:max_waits]
                    extra = waits[max_waits:]
                    # A PE matmul's extra wait can ride its own Ldweights:
                    # the pair reads the same operands and the wait's
                    # producers (other engines' outputs) can never depend
                    # on this Ldweights, so gating the load on the wait is
                    # semantically identical.
                    if isinstance(inst, (mybir.InstMatmult,
                                         mybir.InstLdweights)):
                        # Walk back across the contiguous PE matmul burst
                        # and park extra waits in spare slots. All cross-
                        # engine producers of these waits are >= LAG tiles
                        # old (the epilogue has no matmuls), so gating an
                        # earlier matmul of the same burst cannot deadlock.
                        idx = len(new) - 1
                        steps = 0
                        while extra and idx >= 0 and steps < 8:
                            cand = new[idx]
                            if not isinstance(cand, (mybir.InstMatmult,
                                                     mybir.InstLdweights)):
                                break
                            if cand.engine == inst.engine:
                                csi = cand.sync_info
                                cw = (list(csi.on_wait)
                                      if csi is not None else [])
                                if len(cw) < max_waits:
                                    while len(cw) < max_waits and extra:
                                        cw.append(extra.pop(0))
                                    cand.sync_info = mybir.SyncInfo(
                                        on_wait=cw,
                                        on_update=list(csi.on_update)
                                        if csi
                                        else [],
                                    )
                            idx -= 1
                            steps += 1
                    for w in extra:
                        nop = make_nop(inst.engine)
                        n += 1
                        nop.sync_info = mybir.SyncInfo(
                            on_wait=[w], on_update=[]
                        )
                        new.append(nop)
                    inst.sync_info = mybir.SyncInfo(
                        on_wait=keep, on_update=list(si.on_update)
                    )
                new.append(inst)
            b.instructions[:] = new
    return n


def _segment_ids(scope):
    marks = np.zeros(N_SENT, np.int64)
    np.add.at(marks, scope[1:-1].astype(np.int64), 1)
    return np.cumsum(marks)


def _balanced_chunks(counts):
    """Partition bags into <=NCHUNK contiguous chunks, <=64 bags each,
    minimizing the max sentence count per chunk. Returns list of (b0, b1)."""
    total = int(counts.sum())

    def greedy(cap):
        bounds = []
        s = 0
        n = 0
        b0 = 0
        for b in range(N_BAGS):
            c = int(counts[b])
            pos = len(bounds) % CHUNKS_PER_CORE
            cap_k = cap // 3 if pos == CHUNKS_PER_CORE - 1 else cap
            if n == MAX_BAGS_PER_CHUNK or (s + c > cap_k and n > 0):
                bounds.append((b0, b))
                b0 = b
                s = 0
                n = 0
            s += c
            n += 1
        bounds.append((b0, N_BAGS))
        return bounds

    lo = max(int(counts.max()), total // NCHUNK)
    hi = total
    while lo < hi:
        mid = (lo + hi) // 2
        if len(greedy(mid)) <= NCHUNK:
            hi = mid
        else:
            lo = mid + 1
    bounds = greedy(lo)
    while len(bounds) < NCHUNK:
        bounds.append((N_BAGS, N_BAGS))
    return bounds


def _build_bass(tcs, split=True):
    import concourse.mybir as mybir
    from concourse import bass
    from concourse.tile import TileContext

    _patch_tile_drain()
    f32 = mybir.dt.float32
    bf16 = mybir.dt.bfloat16
    AO = mybir.AluOpType
    Act = mybir.ActivationFunctionType
    CH = CHUNKS_PER_CORE
    Tc = tcs[0]

    nc = bass.Bass("TRN2")
    d_xq = nc.dram_tensor("xq", [128, CH * 2 * Tc * 128], bf16, kind="ExternalInput")
    d_mt = nc.dram_tensor("mt", [128, CH * 3 * Tc], f32, kind="ExternalInput")
    d_cb = nc.dram_tensor("cb", [128, 478], bf16, kind="ExternalInput")
    d_out = nc.dram_tensor("out", [64, CH * 106], f32, kind="ExternalOutput")

    with TileContext(nc) as tc:
        with (
            tc.tile_pool(name="const", bufs=1) as cpool,
            tc.tile_pool(name="xqp", bufs=3) as xqp,
            tc.tile_pool(name="mtp", bufs=3) as mtp,
            tc.tile_pool(name="ep", bufs=5) as epool,
            tc.tile_pool(name="psbp", bufs=7) as psbp,
            tc.tile_pool(name="scrp", bufs=24) as scrp,
            tc.tile_pool(name="etp", bufs=8) as etp,
            tc.tile_pool(name="a2p", bufs=8) as a2pool,
            tc.tile_pool(name="miscp", bufs=4) as miscp,
            tc.tile_pool(name="ps_flt", bufs=2, space="PSUM") as ps_flt,
            tc.tile_pool(name="ps_p", bufs=2, space="PSUM") as ps_p,
            tc.tile_pool(name="ps_o", bufs=2, space="PSUM") as ps_o,
            tc.tile_pool(name="ps_dn", bufs=1, space="PSUM") as ps_dn,
        ):
            cb = cpool.tile([128, 478], bf16, tag="cb")
            nc.sync.dma_start(out=cb[:], in_=d_cb[:])
            io14 = cb[:, 0:14]
            io53 = cb[:, 14:67]
            io64 = cb[:, 67:131]
            ones1 = cb[:, 131:132]
            cc = cb[:, 132:266]       # [128, 2*67] logit weights, per half
            dct = cb[:, 266:478]      # [128, 2*106] disc^T, per half
            pending_epi = []

            for k in range(CH):
                nt = tcs[k]
                xq = xqp.tile([128, 2 * Tc * 128], bf16, tag="xq")
                mt = mtp.tile([128, 3 * Tc], f32, tag="mt")
                nc.sync.dma_start(
                    out=mt[:, 0 : 3 * nt],
                    in_=d_mt[:, k * 3 * Tc : k * 3 * Tc + 3 * nt],
                )
                cuts = sorted(set(
                    nt * i // DMA_SPLIT for i in range(DMA_SPLIT + 1)
                ))
                if k == 0:
                    # small leading piece fills the pipeline fast at start
                    cuts = sorted(set([0, min(2, nt)] + cuts))
                for i in range(len(cuts) - 1):
                    t0, t1 = cuts[i], cuts[i + 1]
                    if t1 == t0:
                        continue
                    for hh in range(2):
                        nc.sync.dma_start(
                            out=xq[:, hh * Tc * 128 + t0 * 128
                                   : hh * Tc * 128 + t1 * 128],
                            in_=d_xq[:, (k * 2 + hh) * Tc * 128 + t0 * 128
                                     : (k * 2 + hh) * Tc * 128 + t1 * 128],
                        )

                if len(pending_epi) >= EPI_DEPTH:
                    pending_epi.pop(0)()

                O = ps_o.tile([128, 106], f32, tag="O", name="O")
                dn = ps_dn.tile([128, 1], f32, tag="dn", name="dn")

                tail_state = {"et": None, "a2": None}
                NG_E = (nt + EXP_GROUP - 1) // EXP_GROUP
                NG_P = (nt + P_GROUP - 1) // P_GROUP
                Es = [None] * NG_E
                Ps = [None] * NG_P
                flt_ps = [None] * NG_E
                p_ps = [None] * NG_P

                def tile_tail(j, Es=Es, Ps=Ps, mt=mt, O=O, dn=dn, nt=nt):
                    ge, jj_e = j // EXP_GROUP, j % EXP_GROUP
                    gp, jj_p = j // P_GROUP, j % P_GROUP
                    E = Es[ge]
                    Psb = Ps[gp]
                    lbl1 = mt[:, 3 * j + 0 : 3 * j + 1]
                    lbl0 = mt[:, 3 * j + 1 : 3 * j + 2]
                    sg = mt[:, 3 * j + 2 : 3 * j + 3]
                    sj = j % TAILB
                    if tail_state["et"] is None or sj == 0:
                        tail_state["et"] = etp.tile(
                            [128, 2 * TAILB], f32, tag="et", name="etb"
                        )
                        tail_state["a2"] = a2pool.tile(
                            [128, 128 * TAILB], bf16, tag="a2", name="a2b"
                        )
                    et = tail_state["et"][:, 2 * sj : 2 * sj + 2]
                    a2 = tail_state["a2"][:, 128 * sj : 128 * (sj + 1)]
                    scr0 = scrp.tile([128, 14], bf16, tag="scr0")
                    scr1 = scrp.tile([128, 53], bf16, tag="scr1")
                    # et1 = E[s, 14+lbl1]  (DVE; Pool stt unsupported by walrus)
                    nc.vector.scalar_tensor_tensor(
                        scr1[:], io53, lbl1,
                        E[:, jj_e * 67 + 14 : jj_e * 67 + 67],
                        AO.is_equal, AO.mult, accum_out=et[:, 1:2],
                    )
                    # et0 = E[s, lbl0]  (DVE stt is_equal/mult + accum)
                    nc.vector.scalar_tensor_tensor(
                        scr0[:], io14, lbl0,
                        E[:, jj_e * 67 : jj_e * 67 + 14],
                        AO.is_equal, AO.mult, accum_out=et[:, 0:1],
                    )
                    nc.vector.tensor_scalar(
                        a2[:, 0:64], io64, sg, et[:, 0:1], AO.is_equal, AO.mult
                    )
                    nc.gpsimd.tensor_scalar(
                        a2[:, 64:128], io64, sg, et[:, 1:2], AO.is_equal, AO.mult
                    )
                    st = j == 0
                    sp = j == nt - 1
                    nc.tensor.matmul(
                        O[:], a2[:], Psb[:, jj_p * 106 : (jj_p + 1) * 106],
                        start=st, stop=sp,
                    )
                    nc.tensor.matmul(dn[:], a2[:], ones1, start=st, stop=sp)

                for j in range(nt):
                    ge, jj_e = j // EXP_GROUP, j % EXP_GROUP
                    gp, jj_p = j // P_GROUP, j % P_GROUP
                    if jj_e == 0:
                        flt_ps[ge] = ps_flt.tile(
                            [128, EXP_GROUP * 67], f32, tag="flt",
                            name="flt_g",
                        )
                    if jj_p == 0:
                        p_ps[gp] = ps_p.tile([128, P_GROUP * 106], f32, tag="pp",
                                             name="pp_g")
                    flt = flt_ps[ge]
                    pp = p_ps[gp]
                    of = flt[:, jj_e * 67 : (jj_e + 1) * 67]
                    op = pp[:, jj_p * 106 : (jj_p + 1) * 106]
                    for hh in range(2):
                        xqj = xq[:, (hh * Tc + j) * 128 : (hh * Tc + j + 1) * 128]
                        nc.tensor.matmul(
                            of, xqj, cc[:, hh * 67 : (hh + 1) * 67],
                            start=(hh == 0), stop=(hh == 1),
                        )
                    for hh in range(2):
                        xqj = xq[:, (hh * Tc + j) * 128 : (hh * Tc + j + 1) * 128]
                        nc.tensor.matmul(
                            op, xqj, dct[:, hh * 106 : (hh + 1) * 106],
                            start=(hh == 0), stop=(hh == 1),
                        )
                    if jj_e == EXP_GROUP - 1 or j == nt - 1:
                        w = (jj_e + 1) * 67
                        E = epool.tile([128, EXP_GROUP * 67], bf16, tag="E",
                                       name="E_g")
                        nc.scalar.activation(E[:, 0:w], flt[:, 0:w], Act.Exp)
                        Es[ge] = E
                    if jj_p == P_GROUP - 1 or j == nt - 1:
                        w = (jj_p + 1) * 106
                        Psb = psbp.tile([128, P_GROUP * 106], bf16, tag="Psb",
                                        name="Psb_g")
                        nc.scalar.activation(Psb[:, 0:w], pp[:, 0:w], Act.Copy)
                        Ps[gp] = Psb
                    if j >= LAG:
                        tile_tail(j - LAG)
                for j in range(max(nt - LAG, 0), nt):
                    tile_tail(j)

                def make_epilogue(k=k, O=O, dn=dn):
                    def epi():
                        sden = miscp.tile([128, 1], f32, tag="sden")
                        inv = miscp.tile([128, 1], f32, tag="inv")
                        nc.vector.tensor_scalar(
                            sden[:], dn[:], 1e-30, None, AO.add
                        )
                        nc.vector.reciprocal(inv[:], sden[:])
                        obo = miscp.tile([128, 106], f32, tag="obo")
                        nc.scalar.activation(
                            obo[:], O[:], Act.Copy, scale=inv[:]
                        )
                        nc.sync.dma_start(
                            out=d_out[:, k * 106 : k * 106 + 53],
                            in_=obo[0:64, 0:53],
                        )
                        nc.sync.dma_start(
                            out=d_out[:, k * 106 + 53 : (k + 1) * 106],
                            in_=obo[64:128, 53:106],
                        )
                    return epi

                pending_epi.append(make_epilogue())
            for epi in pending_epi:
                epi()
            pending_epi.clear()

    if split:
        _split_all_waits(nc)
    return nc


def _prep(x, rel_emb0, rel_emb1, disc, bias, relation_levels, label_index, scope):
    import concourse.mybir as mybir

    bf = mybir.dt.np(mybir.dt.bfloat16)
    CH = CHUNKS_PER_CORE
    seg = _segment_ids(np.asarray(scope))
    counts = np.bincount(seg, minlength=N_BAGS).astype(np.int64)
    cum = np.concatenate([[0], np.cumsum(counts)])
    bounds = _balanced_chunks(counts)
    newbounds = []
    for c in range(NCORE):
        grp = bounds[c * CH : (c + 1) * CH]
        bigc = sorted(grp[: CH - 1],
                      key=lambda bb: -(cum[bb[1]] - cum[bb[0]]))
        newbounds += bigc + [grp[CH - 1]]
    bounds = newbounds
    tiles = [[max(1, (int(cum[b1] - cum[b0]) + 127) // 128)
              for (b0, b1) in bounds[c * CH : (c + 1) * CH]]
             for c in range(NCORE)]
    profile = tuple(max(tiles[c][k] for c in range(NCORE))
                    for k in range(CH))
    Tc = profile[0]
    Narr = CH * Tc * 128

    x = np.asarray(x, np.float32)
    labels = np.asarray(label_index, np.int64)
    xbf = x.astype(bf)

    rl = np.asarray(relation_levels, np.int64)
    c0 = np.asarray(rel_emb0, np.float32)            # [14, 256]
    c1 = np.asarray(rel_emb1, np.float32)[rl[:, 1]]  # [53, 256]
    ccat = np.concatenate([c0, c1], 0)               # [67, 256]
    ccT = np.ascontiguousarray(ccat.T)               # [256, 67]
    ccsb = np.zeros((128, 134), np.float32)
    ccsb[:, 0:67] = ccT[0:128]
    ccsb[:, 67:134] = ccT[128:256]

    disc = np.asarray(disc, np.float32)              # [53, 512]
    # dct[:, hh*106 + l*53 + c] = disc[c, l*256 + hh*128 + h']
    dctsb = np.zeros((128, 212), np.float32)
    for hh in range(2):
        for l in range(2):
            dctsb[:, hh * 106 + l * 53 : hh * 106 + (l + 1) * 53] = (
                disc[:, l * 256 + hh * 128 : l * 256 + (hh + 1) * 128].T
            )

    cbsb = np.zeros((128, 478), np.float32)
    cbsb[:, 0:14] = np.arange(14, dtype=np.float32)[None, :]
    cbsb[:, 14:67] = np.arange(53, dtype=np.float32)[None, :]
    cbsb[:, 67:131] = np.arange(64, dtype=np.float32)[None, :]
    cbsb[:, 131] = 1.0
    cbsb[:, 132:266] = ccsb
    cbsb[:, 266:478] = dctsb
    const = {"cb": cbsb.astype(bf)}

    in_maps = []
    meta = []
    for core in range(NCORE):
        xa = np.zeros((Narr, 256), bf)
        m5 = np.zeros((Narr, 3), np.float32)
        m5[:, 0] = 60.0   # lbl1 sentinel (no match)
        m5[:, 1] = 20.0   # lbl0 sentinel (no match)
        m5[:, 2] = 120.0  # sg sentinel
        cmeta = []
        for k in range(CH):
            b0, b1 = bounds[core * CH + k]
            s0, s1 = int(cum[b0]), int(cum[b1])
            L = s1 - s0
            off = k * Tc * 128
            if L > 0:
                xa[off : off + L] = xbf[s0:s1]
                m5[off : off + L, 0] = labels[s0:s1].astype(np.float32)
                m5[off : off + L, 1] = rl[labels[s0:s1], 0].astype(np.float32)
                m5[off : off + L, 2] = (seg[s0:s1] - b0).astype(np.float32)
            cmeta.append((b0, b1))
        meta.append(cmeta)
        # xq[p, ((k*2+hh)*Tc + j)*128 + c] = xa[(k*Tc+j)*128 + c, hh*128 + p]
        xq = np.ascontiguousarray(
            xa.reshape(CH, Tc, 128, 2, 128).transpose(4, 0, 3, 1, 2)
        ).reshape(128, CH * 2 * Tc * 128)
        # mt[p, k*3*Tc + j*3 + t] = m5[(k*Tc+j)*128 + p, t]
        mtar = np.ascontiguousarray(
            m5.reshape(CH, Tc, 128, 3).transpose(2, 0, 1, 3)
        ).reshape(128, CH * 3 * Tc)
        in_maps.append({"xq": xq, "mt": mtar, **const})
    return profile, in_maps, meta


def kernel(x, rel_emb0, rel_emb1, disc, bias, relation_levels, label_index,
           scope, _trace=False):
    from concourse.bass_utils import run_bass_kernel_spmd

    key, in_maps, meta = _prep(
        x, rel_emb0, rel_emb1, disc, bias, relation_levels, label_index, scope
    )
    if key not in _CACHE:
        _CACHE[key] = _build_bass(key)
    nc = _CACHE[key]
    res = None
    for attempt in range(3):
        try:
            res = run_bass_kernel_spmd(
                nc, in_maps, core_ids=list(range(NCORE)), trace=_trace
            )
            break
        except Exception:
            if attempt == 2:
                raise
    bias_np = np.asarray(bias, np.float32)
    out = np.zeros((N_BAGS, NCLS), np.float32)
    for core in range(NCORE):
        o = np.asarray(res.results[core]["out"]).reshape(
            64, CHUNKS_PER_CORE, 106
        )
        for k, (b0, b1) in enumerate(meta[core]):
            if b1 > b0:
                out[b0:b1] = (
                    o[: b1 - b0, k, 0:53]
                    + o[: b1 - b0, k, 53:106]
                    + bias_np[None, :]
                )
    kernel._last_results = res
    return out
